# revision 19
# baseline (speedup 1.0000x reference)
"""Stochastic-computing bitstream AND-popcount kernel for 8 Trainium2 NeuronCores.

Reference computation:
    wbits[o,d,n] = (ranks[o,d,n] < round(clip(kernel[o,d],0,1)*128))   # fixed PRNG ranks
    out[b,o,n]   = (sum_d wbits[o,d,n] * inputs[b,d,n]) / 128

`ranks` depends only on jax.random.key(42) and the shapes, so it is a host
constant. The device work is 128 independent (64x1024)x(1024x512) matmuls
(one per bitstream position n), sharded over 8 cores by n (16 each).
The kernel is HBM-bound: ~10.4 MiB/core of fp8 operands + fp16 results at
~358 GB/s/core ~= 29 us; the tensor engine (~14 us in fp8 with column-paired
matmuls) and PSUM eviction hide under the DMA stream.

Per-core device program (SPMD, no collectives):
  - x  : [128(dm), 16(nn), 8(dc), 64(b)]        fp8 (0.0/1.0)  ~1 MiB
  - w  : [128(dm), 8(pair), 8(dc), 2(par), 512] fp8 (0.0/1.0)  ~8.4 MiB
  - y  : [8(pair), 128(2x64 b), 512(o)]         fp16           ~1 MiB
  For each pair of n-positions: 8 contraction matmuls per n accumulate into
  one PSUM bank; the two n streams target array column halves (psum partitions
  0:64 / 64:128) so they execute concurrently on different PE column groups.
  The w layout puts (pair, dc) outermost so any dc-range chunk is contiguous
  per partition: the first/last pair's loads split into small chunks at full
  DMA efficiency, which starts compute early and leaves only ~2 matmul slots
  + eviction + store after the last byte lands.
"""

import os
import tempfile

import numpy as np
import ml_dtypes

B, D, N, O = 64, 1024, 128, 512
NCORES = 8
NPC = N // NCORES  # n positions per core
DC = D // 128      # contraction chunks of 128
PAIRS = NPC // 2

FP8_ONE = np.uint8(0x38)  # 1.0 in float8_e4m3

_RANKS_CACHE = os.path.join(tempfile.gettempdir(), "bitstream_ranks_pairdc_v2.npy")

_ranks_t = None    # [128(dm), 64(pair), 8(dc), 2(parity), 512(o)] int8
_program = None    # compiled Bass program (module-level cache)

# w DMA chunks as (pair, dc_lo, dc_hi): fine-grained at the stream head (early
# compute start, HAM stays warm) and tail (minimal work after the last byte)
if os.environ.get("BITSTREAM_UNIFORM_CHUNKS"):
    W_CHUNKS = [(p, 0, 8) for p in range(PAIRS)]
else:
    W_CHUNKS = [
        (0, 0, 8),
        (1, 0, 8),
        (2, 0, 8),
        (3, 0, 8),
        (4, 0, 8),
        (5, 0, 8),
        (6, 0, 8),
        (7, 0, 5),
        (7, 5, 7),
        (7, 7, 8),
    ]


def _chunk_thresholds():
    """(pair, dc) -> number of w chunks that must be complete before the
    matmuls for that dc can run."""
    th = {}
    for idx, (p, lo, hi) in enumerate(W_CHUNKS):
        for dc in range(lo, hi):
            th[(p, dc)] = idx + 1
    return th


def _get_ranks_t():
    """Rank-of-each-position for the fixed key(42) permutations, pre-transposed
    to the device layout [dm, pair, dc, parity, o]. Constant across calls."""
    global _ranks_t
    if _ranks_t is not None:
        return _ranks_t
    if os.path.exists(_RANKS_CACHE):
        try:
            r = np.load(_RANKS_CACHE)
            if r.shape == (128, N // 2, DC, 2, O) and r.dtype == np.int8:
                _ranks_t = r
                return _ranks_t
        except Exception:
            pass
    import jax
    import jax.numpy as jnp

    with jax.default_device(jax.devices("cpu")[0]):
        u = jax.random.uniform(jax.random.key(42), (O, D, N))
        perm = np.asarray(jnp.argsort(u, axis=-1))
    # ranks = argsort(argsort(u)) == inverse permutation of argsort(u)
    ranks = np.empty((O, D, N), np.int8)
    np.put_along_axis(
        ranks, perm, np.broadcast_to(np.arange(N, dtype=np.int8), (O, D, N)), axis=-1
    )
    r = ranks.reshape(O, DC, 128, N // 2, 2).transpose(2, 3, 1, 4, 0)
    _ranks_t = np.ascontiguousarray(r)
    try:
        np.save(_RANKS_CACHE, _ranks_t)
    except Exception:
        pass
    return _ranks_t


def _build_program_raw():
    """Hand-scheduled program: no framework barriers, explicit DMA pacing.

    Engine plan (per core):
      SYNC   triggers w DMAs (HWDGE ring) with two chunks in flight so the
             stream stays saturated end to end.
      SCALAR triggers the x DMA on its own HWDGE ring, then the per-pair y
             output DMAs as evictions complete.
      TENSOR warms the PE (HAM clock gate) on dummy operands while the first
             chunks land, then per pair runs the two n-streams interleaved on
             column halves of the array, accumulating over the 8 contraction
             chunks in one PSUM bank (7-bank rotation + 1 scratch).
      VECTOR evicts PSUM -> fp16 SBUF with the 1/128 scale fused.
    """
    from contextlib import ExitStack

    import concourse.bass as bass
    import concourse.mybir as mybir

    fp8 = mybir.dt.float8e4
    fp32 = mybir.dt.float32
    fp16 = mybir.dt.float16
    nc = bass.Bass(target_bir_lowering=False)

    x_d = nc.dram_tensor("x", [128, NPC, DC, B], fp8, kind="ExternalInput")
    w_d = nc.dram_tensor("w", [128, PAIRS, DC, 2, O], fp8, kind="ExternalInput")
    y_d = nc.dram_tensor("y", [PAIRS, 128, O], fp16, kind="ExternalOutput")

    NBANK = 7
    WARMUP_MMS = 26
    thresholds = _chunk_thresholds()

    with (
        ExitStack() as stack,
        nc.sbuf_tensor([128, NPC, DC, B], fp8) as x_sb,
        nc.sbuf_tensor([128, PAIRS, DC, 2, O], fp8) as w_sb,
        nc.sbuf_tensor([128, PAIRS, O], fp16) as o_sb,
        nc.sbuf_tensor([128, B], fp8) as dum_x,
        nc.sbuf_tensor([128, O], fp8) as dum_w,
        nc.psum_tensor([128, NBANK, O], fp32) as ps,
        nc.psum_tensor([128, O], fp32) as ps_scratch,
        nc.semaphore("x_sem") as x_sem,
        nc.semaphore("mm_sem") as mm_sem,
        nc.semaphore("evac_sem") as evac_sem,
        nc.semaphore("y_sem") as y_sem,
        nc.Block() as block,
    ):
        # one semaphore per w chunk: chunks differ in size, so completions on
        # the shared ring can arrive out of order — a cumulative count would
        # unblock matmuls whose own chunk hasn't landed yet
        w_sems = [
            stack.enter_context(nc.semaphore(f"w_sem{k}"))
            for k in range(len(W_CHUNKS))
        ]

        @block.sync
        def _(sync: bass.BassEngine):
            for k, (p, lo, hi) in enumerate(W_CHUNKS):
                if k == 2:
                    # x rides behind the first two w chunks: pair0's weights
                    # land earlier and x is still resident long before pair0
                    # can run
                    sync.dma_start(out=x_sb[:], in_=x_d[:]).then_inc(x_sem, 16)
                if k >= 2:
                    sync.wait_ge(w_sems[k - 2], 16)
                sync.dma_start(
                    out=w_sb[:, p, lo:hi], in_=w_d[:, p, lo:hi]
                ).then_inc(w_sems[k], 16)
            sync.wait_ge(y_sem, 16 * PAIRS)

        @block.scalar
        def _(scalar: bass.BassEngine):
            for p in range(PAIRS):
                scalar.wait_ge(evac_sem, p + 1)
                scalar.dma_start(out=y_d[p], in_=o_sb[:, p, :]).then_inc(y_sem, 16)

        @block.tensor
        def _(tensor: bass.BassEngine):
            # HAM warmup on garbage data while input DMAs stream
            for i in range(WARMUP_MMS):
                tensor.matmul(
                    ps_scratch[0:64, :],
                    dum_x[:, 0:64],
                    dum_w[:],
                    start=(i == 0),
                    stop=(i == WARMUP_MMS - 1),
                )
            tensor.wait_ge(x_sem, 16)
            for p in range(PAIRS):
                if p >= NBANK:
                    tensor.wait_ge(evac_sem, p - NBANK + 1)
                bank = p % NBANK
                last = None
                cur_th = 0
                for dc in range(DC):
                    if thresholds[(p, dc)] > cur_th:
                        cur_th = thresholds[(p, dc)]
                        tensor.wait_ge(w_sems[cur_th - 1], 16)
                    tensor.matmul(
                        ps[0:64, bank, :],
                        x_sb[:, 2 * p, dc, :],
                        w_sb[:, p, dc, 0, :],
                        start=(dc == 0),
                        stop=(dc == DC - 1),
                    )
                    last = tensor.matmul(
                        ps[64:128, bank, :],
                        x_sb[:, 2 * p + 1, dc, :],
                        w_sb[:, p, dc, 1, :],
                        start=(dc == 0),
                        stop=(dc == DC - 1),
                    )
                last.then_inc(mm_sem, 1)

        @block.vector
        def _(vector: bass.BassEngine):
            for p in range(PAIRS):
                vector.wait_ge(mm_sem, p + 1)
                vector.tensor_scalar_mul(
                    o_sb[:, p, :], ps[:, p % NBANK, :], 1.0 / N
                ).then_inc(evac_sem, 1)

    return nc


def _build_program():
    global _program
    if _program is None:
        _program = _build_program_raw()
    return _program


def _prep_inputs(inputs, kernel):
    """Full inputs -> per-core in_maps (fp8 device layouts)."""
    ranks_t = _get_ranks_t()

    # weight bitstreams, directly in device layout [dm, pair, dc, parity, o]
    nb = np.round(np.clip(kernel, 0.0, 1.0) * np.float32(N)).astype(np.int16)  # (O, D)
    nb_m1 = (nb - 1).astype(np.int8).reshape(O, DC, 128).transpose(2, 1, 0)  # [dm,dc,o]
    wb = ranks_t <= nb_m1[:, None, :, None, :]  # bool [dm, pair, dc, parity, o]
    w8 = wb.view(np.uint8) * FP8_ONE

    # input bitstreams [dm, n, dc, b]
    x8 = (inputs.astype(np.uint8) * FP8_ONE).reshape(B, DC, 128, N).transpose(2, 3, 1, 0)

    in_maps = []
    for i in range(NCORES):
        in_maps.append(
            {
                "x": np.ascontiguousarray(x8[:, NPC * i : NPC * (i + 1)]).view(
                    ml_dtypes.float8_e4m3
                ),
                "w": np.ascontiguousarray(w8[:, PAIRS * i : PAIRS * (i + 1)]).view(
                    ml_dtypes.float8_e4m3
                ),
            }
        )
    return in_maps


def _assemble_output(results):
    out = np.empty((B, O, N), np.float32)
    for i, res in enumerate(results):
        y = np.asarray(res["y"])  # (PAIRS, 128, O) fp16
        out[:, :, NPC * i : NPC * (i + 1)] = (
            y.reshape(PAIRS, 2, B, O).transpose(2, 3, 0, 1).reshape(B, O, NPC)
        )
    return out


def run(inputs, kernel, trace=False):
    """Returns (output, BassKernelResults)."""
    from concourse.bass_utils import run_bass_kernel_spmd

    nc = _build_program()
    in_maps = _prep_inputs(np.asarray(inputs), np.asarray(kernel))
    bres = run_bass_kernel_spmd(nc, in_maps, list(range(NCORES)), trace=trace)
    return _assemble_output(bres.results), bres


def kernel(inputs, kernel):
    out, _ = run(inputs, kernel)
    return out


# revision 22
# speedup vs baseline: 1.0761x; 1.0761x over previous
"""Stochastic-computing bitstream AND-popcount kernel for 8 Trainium2 NeuronCores.

Reference computation:
    wbits[o,d,n] = (ranks[o,d,n] < round(clip(kernel[o,d],0,1)*128))   # fixed PRNG ranks
    out[b,o,n]   = (sum_d wbits[o,d,n] * inputs[b,d,n]) / 128

`ranks` depends only on jax.random.key(42) and the shapes, so it is a host
constant. The device work is 128 independent (64x1024)x(1024x512) matmuls
(one per bitstream position n), sharded over 8 cores by n (16 each).
The kernel is HBM-bound: ~10.4 MiB/core of fp8 operands + fp16 results at
~358 GB/s/core ~= 29 us; the tensor engine (~14 us in fp8 with column-paired
matmuls) and PSUM eviction hide under the DMA stream.

Per-core device program (SPMD, no collectives):
  - x  : [128(dm), 16(nn), 8(dc), 64(b)]        fp8 (0.0/1.0)  ~1 MiB
  - w  : [128(dm), 8(pair), 8(dc), 2(par), 512] fp8 (0.0/1.0)  ~8.4 MiB
  - y  : [8(pair), 128(2x64 b), 512(o)]         fp16           ~1 MiB
  For each pair of n-positions: 8 contraction matmuls per n accumulate into
  one PSUM bank; the two n streams target array column halves (psum partitions
  0:64 / 64:128) so they execute concurrently on different PE column groups.
  The w layout puts (pair, dc) outermost so any dc-range chunk is contiguous
  per partition: the first/last pair's loads split into small chunks at full
  DMA efficiency, which starts compute early and leaves only ~2 matmul slots
  + eviction + store after the last byte lands.
"""

import os
import tempfile

import numpy as np
import ml_dtypes

B, D, N, O = 64, 1024, 128, 512
NCORES = 8
NPC = N // NCORES  # n positions per core
DC = D // 128      # contraction chunks of 128
PAIRS = NPC // 2

FP8_ONE = np.uint8(0x38)  # 1.0 in float8_e4m3

_RANKS_CACHE = os.path.join(tempfile.gettempdir(), "bitstream_ranks_pairdc_v2.npy")

_ranks_t = None    # [128(dm), 64(pair), 8(dc), 2(parity), 512(o)] int8
_program = None    # compiled Bass program (module-level cache)

# w DMA chunks as (pair, dc_lo, dc_hi): fine-grained at the stream head (early
# compute start, HAM stays warm) and tail (minimal work after the last byte)
if os.environ.get("BITSTREAM_UNIFORM_CHUNKS"):
    W_CHUNKS = [(p, 0, 8) for p in range(PAIRS)]
else:
    W_CHUNKS = [
        (0, 0, 8),
        (1, 0, 8),
        (2, 0, 8),
        (3, 0, 8),
        (4, 0, 8),
        (5, 0, 8),
        (6, 0, 8),
        (7, 0, 5),
        (7, 5, 7),
        (7, 7, 8),
    ]


def _chunk_thresholds():
    """(pair, dc) -> number of w chunks that must be complete before the
    matmuls for that dc can run."""
    th = {}
    for idx, (p, lo, hi) in enumerate(W_CHUNKS):
        for dc in range(lo, hi):
            th[(p, dc)] = idx + 1
    return th


def _get_ranks_t():
    """Rank-of-each-position for the fixed key(42) permutations, pre-transposed
    to the device layout [dm, pair, dc, parity, o]. Constant across calls."""
    global _ranks_t
    if _ranks_t is not None:
        return _ranks_t
    if os.path.exists(_RANKS_CACHE):
        try:
            r = np.load(_RANKS_CACHE)
            if r.shape == (128, N // 2, DC, 2, O) and r.dtype == np.int8:
                _ranks_t = r
                return _ranks_t
        except Exception:
            pass
    import jax
    import jax.numpy as jnp

    with jax.default_device(jax.devices("cpu")[0]):
        u = jax.random.uniform(jax.random.key(42), (O, D, N))
        perm = np.asarray(jnp.argsort(u, axis=-1))
    # ranks = argsort(argsort(u)) == inverse permutation of argsort(u)
    ranks = np.empty((O, D, N), np.int8)
    np.put_along_axis(
        ranks, perm, np.broadcast_to(np.arange(N, dtype=np.int8), (O, D, N)), axis=-1
    )
    r = ranks.reshape(O, DC, 128, N // 2, 2).transpose(2, 3, 1, 4, 0)
    _ranks_t = np.ascontiguousarray(r)
    try:
        np.save(_RANKS_CACHE, _ranks_t)
    except Exception:
        pass
    return _ranks_t


def _build_program_raw():
    """Hand-scheduled program: no framework barriers, explicit DMA pacing.

    Engine plan (per core):
      SYNC   triggers w DMAs (HWDGE ring) with two chunks in flight so the
             stream stays saturated end to end.
      SCALAR triggers the x DMA on its own HWDGE ring, then the per-pair y
             output DMAs as evictions complete.
      TENSOR warms the PE (HAM clock gate) on dummy operands while the first
             chunks land, then per pair runs the two n-streams interleaved on
             column halves of the array, accumulating over the 8 contraction
             chunks in one PSUM bank (7-bank rotation + 1 scratch).
      VECTOR evicts PSUM -> fp16 SBUF with the 1/128 scale fused.
    """
    from contextlib import ExitStack

    import concourse.bass as bass
    import concourse.mybir as mybir

    fp8 = mybir.dt.float8e4
    fp32 = mybir.dt.float32
    fp16 = mybir.dt.float16
    nc = bass.Bass(target_bir_lowering=False)

    x_d = nc.dram_tensor("x", [128, NPC, DC, B], fp8, kind="ExternalInput")
    w_d = nc.dram_tensor("w", [128, PAIRS, DC, 2, O], fp8, kind="ExternalInput")
    y_d = nc.dram_tensor("y", [PAIRS, 128, O], fp16, kind="ExternalOutput")

    NBANK = 7
    WARMUP_MMS = 26
    thresholds = _chunk_thresholds()

    with (
        ExitStack() as stack,
        nc.sbuf_tensor([128, NPC, DC, B], fp8) as x_sb,
        nc.sbuf_tensor([128, PAIRS, DC, 2, O], fp8) as w_sb,
        nc.sbuf_tensor([128, PAIRS, O], fp16) as o_sb,
        nc.sbuf_tensor([128, B], fp8) as dum_x,
        nc.sbuf_tensor([128, O], fp8) as dum_w,
        nc.psum_tensor([128, NBANK, O], fp32) as ps,
        nc.psum_tensor([128, O], fp32) as ps_scratch,
        nc.semaphore("x_sem") as x_sem,
        nc.semaphore("mm_sem") as mm_sem,
        nc.semaphore("evac_sem") as evac_sem,
        nc.semaphore("evac_h_sem") as evac_h_sem,
        nc.semaphore("y_sem") as y_sem,
        nc.Block() as block,
    ):
        # one semaphore per w chunk: chunks differ in size, so completions on
        # the shared ring can arrive out of order — a cumulative count would
        # unblock matmuls whose own chunk hasn't landed yet
        w_sems = [
            stack.enter_context(nc.semaphore(f"w_sem{k}"))
            for k in range(len(W_CHUNKS))
        ]

        @block.sync
        def _(sync: bass.BassEngine):
            for k, (p, lo, hi) in enumerate(W_CHUNKS):
                if k == 2:
                    # x rides behind the first two w chunks: pair0's weights
                    # land earlier and x is still resident long before pair0
                    # can run
                    sync.dma_start(out=x_sb[:], in_=x_d[:]).then_inc(x_sem, 16)
                if k >= 2:
                    sync.wait_ge(w_sems[k - 2], 16)
                sync.dma_start(
                    out=w_sb[:, p, lo:hi], in_=w_d[:, p, lo:hi]
                ).then_inc(w_sems[k], 16)
            sync.wait_ge(y_sem, 16 * (PAIRS + 1))

        @block.scalar
        def _(scalar: bass.BassEngine):
            for p in range(PAIRS - 1):
                scalar.wait_ge(evac_sem, p + 1)
                scalar.dma_start(out=y_d[p], in_=o_sb[:, p, :]).then_inc(y_sem, 16)
            # last pair streams out in column halves, overlapping the second
            # half's eviction with the first half's store
            last = PAIRS - 1
            scalar.wait_ge(evac_h_sem, 1)
            scalar.dma_start(
                out=y_d[last][:, 0 : O // 2], in_=o_sb[:, last, 0 : O // 2]
            ).then_inc(y_sem, 16)
            scalar.wait_ge(evac_h_sem, 2)
            scalar.dma_start(
                out=y_d[last][:, O // 2 : O], in_=o_sb[:, last, O // 2 : O]
            ).then_inc(y_sem, 16)

        @block.tensor
        def _(tensor: bass.BassEngine):
            # HAM warmup on garbage data while input DMAs stream
            for i in range(WARMUP_MMS):
                tensor.matmul(
                    ps_scratch[0:64, :],
                    dum_x[:, 0:64],
                    dum_w[:],
                    start=(i == 0),
                    stop=(i == WARMUP_MMS - 1),
                )
            tensor.wait_ge(x_sem, 16)
            for p in range(PAIRS):
                if p >= NBANK:
                    tensor.wait_ge(evac_sem, p - NBANK + 1)
                bank = p % NBANK
                last = None
                cur_th = 0
                for dc in range(DC):
                    if thresholds[(p, dc)] > cur_th:
                        cur_th = thresholds[(p, dc)]
                        tensor.wait_ge(w_sems[cur_th - 1], 16)
                    tensor.matmul(
                        ps[0:64, bank, :],
                        x_sb[:, 2 * p, dc, :],
                        w_sb[:, p, dc, 0, :],
                        start=(dc == 0),
                        stop=(dc == DC - 1),
                    )
                    last = tensor.matmul(
                        ps[64:128, bank, :],
                        x_sb[:, 2 * p + 1, dc, :],
                        w_sb[:, p, dc, 1, :],
                        start=(dc == 0),
                        stop=(dc == DC - 1),
                    )
                last.then_inc(mm_sem, 1)

        @block.vector
        def _(vector: bass.BassEngine):
            for p in range(PAIRS - 1):
                vector.wait_ge(mm_sem, p + 1)
                vector.tensor_scalar_mul(
                    o_sb[:, p, :], ps[:, p % NBANK, :], 1.0 / N
                ).then_inc(evac_sem, 1)
            last = PAIRS - 1
            vector.wait_ge(mm_sem, PAIRS)
            vector.tensor_scalar_mul(
                o_sb[:, last, 0 : O // 2],
                ps[:, last % NBANK, 0 : O // 2],
                1.0 / N,
            ).then_inc(evac_h_sem, 1)
            vector.tensor_scalar_mul(
                o_sb[:, last, O // 2 : O],
                ps[:, last % NBANK, O // 2 : O],
                1.0 / N,
            ).then_inc(evac_h_sem, 1)

    return nc


def _build_program():
    global _program
    if _program is None:
        _program = _build_program_raw()
    return _program


def _prep_inputs(inputs, kernel):
    """Full inputs -> per-core in_maps (fp8 device layouts)."""
    ranks_t = _get_ranks_t()

    # weight bitstreams, directly in device layout [dm, pair, dc, parity, o]
    nb = np.round(np.clip(kernel, 0.0, 1.0) * np.float32(N)).astype(np.int16)  # (O, D)
    nb_m1 = (nb - 1).astype(np.int8).reshape(O, DC, 128).transpose(2, 1, 0)  # [dm,dc,o]
    wb = ranks_t <= nb_m1[:, None, :, None, :]  # bool [dm, pair, dc, parity, o]
    w8 = wb.view(np.uint8) * FP8_ONE

    # input bitstreams [dm, n, dc, b]
    x8 = (inputs.astype(np.uint8) * FP8_ONE).reshape(B, DC, 128, N).transpose(2, 3, 1, 0)

    in_maps = []
    for i in range(NCORES):
        in_maps.append(
            {
                "x": np.ascontiguousarray(x8[:, NPC * i : NPC * (i + 1)]).view(
                    ml_dtypes.float8_e4m3
                ),
                "w": np.ascontiguousarray(w8[:, PAIRS * i : PAIRS * (i + 1)]).view(
                    ml_dtypes.float8_e4m3
                ),
            }
        )
    return in_maps


def _assemble_output(results):
    out = np.empty((B, O, N), np.float32)
    for i, res in enumerate(results):
        y = np.asarray(res["y"])  # (PAIRS, 128, O) fp16
        out[:, :, NPC * i : NPC * (i + 1)] = (
            y.reshape(PAIRS, 2, B, O).transpose(2, 3, 0, 1).reshape(B, O, NPC)
        )
    return out


def run(inputs, kernel, trace=False):
    """Returns (output, BassKernelResults)."""
    from concourse.bass_utils import run_bass_kernel_spmd

    nc = _build_program()
    in_maps = _prep_inputs(np.asarray(inputs), np.asarray(kernel))
    bres = run_bass_kernel_spmd(nc, in_maps, list(range(NCORES)), trace=trace)
    return _assemble_output(bres.results), bres


def kernel(inputs, kernel):
    out, _ = run(inputs, kernel)
    return out


# revision 25
# speedup vs baseline: 1.1502x; 1.0689x over previous
"""Stochastic-computing bitstream AND-popcount kernel for 8 Trainium2 NeuronCores.

Reference computation:
    wbits[o,d,n] = (ranks[o,d,n] < round(clip(kernel[o,d],0,1)*128))   # fixed PRNG ranks
    out[b,o,n]   = (sum_d wbits[o,d,n] * inputs[b,d,n]) / 128

`ranks` depends only on jax.random.key(42) and the shapes, so it is a host
constant. The device work is 128 independent (64x1024)x(1024x512) matmuls
(one per bitstream position n), sharded over 8 cores by n (16 each).
The kernel is HBM-bound: ~10.4 MiB/core of fp8 operands + fp16 results at
~358 GB/s/core ~= 29 us; the tensor engine (~14 us in fp8 with column-paired
matmuls) and PSUM eviction hide under the DMA stream.

Per-core device program (SPMD, no collectives):
  - x  : [128(dm), 16(nn), 8(dc), 64(b)]        fp8 (0.0/1.0)  ~1 MiB
  - w  : [128(dm), 8(pair), 8(dc), 2(par), 512] fp8 (0.0/1.0)  ~8.4 MiB
  - y  : [8(pair), 128(2x64 b), 512(o)]         fp16           ~1 MiB
  For each pair of n-positions: 8 contraction matmuls per n accumulate into
  one PSUM bank; the two n streams target array column halves (psum partitions
  0:64 / 64:128) so they execute concurrently on different PE column groups.
  The w layout puts (pair, dc) outermost so any dc-range chunk is contiguous
  per partition: the first/last pair's loads split into small chunks at full
  DMA efficiency, which starts compute early and leaves only ~2 matmul slots
  + eviction + store after the last byte lands.
"""

import os
import tempfile

import numpy as np
import ml_dtypes

B, D, N, O = 64, 1024, 128, 512
NCORES = 8
NPC = N // NCORES  # n positions per core
DC = D // 128      # contraction chunks of 128
PAIRS = NPC // 2

FP8_ONE = np.uint8(0x38)  # 1.0 in float8_e4m3

_RANKS_CACHE = os.path.join(tempfile.gettempdir(), "bitstream_ranks_pairdc_v2.npy")

_ranks_t = None    # [128(dm), 64(pair), 8(dc), 2(parity), 512(o)] int8
_program = None    # compiled Bass program (module-level cache)

# w DMA chunks as (pair, dc_lo, dc_hi): fine-grained at the stream head (early
# compute start, HAM stays warm) and tail (minimal work after the last byte)
if os.environ.get("BITSTREAM_UNIFORM_CHUNKS"):
    W_CHUNKS = [(p, 0, 8) for p in range(PAIRS)]
else:
    W_CHUNKS = [
        (0, 0, 8),
        (1, 0, 8),
        (2, 0, 8),
        (3, 0, 8),
        (4, 0, 8),
        (5, 0, 8),
        (6, 0, 8),
        (7, 0, 5),
        (7, 5, 7),
        (7, 7, 8),
    ]


def _chunk_thresholds():
    """(pair, dc) -> number of w chunks that must be complete before the
    matmuls for that dc can run."""
    th = {}
    for idx, (p, lo, hi) in enumerate(W_CHUNKS):
        for dc in range(lo, hi):
            th[(p, dc)] = idx + 1
    return th


def _get_ranks_t():
    """Rank-of-each-position for the fixed key(42) permutations, pre-transposed
    to the device layout [dm, pair, dc, parity, o]. Constant across calls."""
    global _ranks_t
    if _ranks_t is not None:
        return _ranks_t
    if os.path.exists(_RANKS_CACHE):
        try:
            r = np.load(_RANKS_CACHE)
            if r.shape == (128, N // 2, DC, 2, O) and r.dtype == np.int8:
                _ranks_t = r
                return _ranks_t
        except Exception:
            pass
    import jax
    import jax.numpy as jnp

    with jax.default_device(jax.devices("cpu")[0]):
        u = jax.random.uniform(jax.random.key(42), (O, D, N))
        perm = np.asarray(jnp.argsort(u, axis=-1))
    # ranks = argsort(argsort(u)) == inverse permutation of argsort(u)
    ranks = np.empty((O, D, N), np.int8)
    np.put_along_axis(
        ranks, perm, np.broadcast_to(np.arange(N, dtype=np.int8), (O, D, N)), axis=-1
    )
    r = ranks.reshape(O, DC, 128, N // 2, 2).transpose(2, 3, 1, 4, 0)
    _ranks_t = np.ascontiguousarray(r)
    try:
        np.save(_RANKS_CACHE, _ranks_t)
    except Exception:
        pass
    return _ranks_t


def _build_program_raw():
    """Hand-scheduled program: no framework barriers, explicit DMA pacing.

    Engine plan (per core):
      SYNC   triggers w DMAs (HWDGE ring) with two chunks in flight so the
             stream stays saturated end to end.
      SCALAR triggers the x DMA on its own HWDGE ring, then the per-pair y
             output DMAs as evictions complete.
      TENSOR warms the PE (HAM clock gate) on dummy operands while the first
             chunks land, then per pair runs the two n-streams interleaved on
             column halves of the array, accumulating over the 8 contraction
             chunks in one PSUM bank (7-bank rotation + 1 scratch).
      VECTOR evicts PSUM -> fp16 SBUF with the 1/128 scale fused.
    """
    from contextlib import ExitStack

    import concourse.bass as bass
    import concourse.mybir as mybir

    fp8 = mybir.dt.float8e4
    fp32 = mybir.dt.float32
    fp16 = mybir.dt.float16
    nc = bass.Bass(target_bir_lowering=False)

    x_d = nc.dram_tensor("x", [128, NPC, DC, B], fp8, kind="ExternalInput")
    w_d = nc.dram_tensor("w", [128, PAIRS, DC, 2, O], fp8, kind="ExternalInput")
    y_d = nc.dram_tensor("y", [PAIRS, 128, O], fp16, kind="ExternalOutput")

    NBANK = 7
    WARMUP_MMS = 26
    thresholds = _chunk_thresholds()

    with (
        ExitStack() as stack,
        nc.sbuf_tensor([128, NPC, DC, B], fp8) as x_sb,
        nc.sbuf_tensor([128, PAIRS, DC, 2, O], fp8) as w_sb,
        nc.sbuf_tensor([128, PAIRS, O], fp16) as o_sb,
        nc.sbuf_tensor([128, B], fp8) as dum_x,
        nc.sbuf_tensor([128, O], fp8) as dum_w,
        nc.psum_tensor([128, NBANK, O], fp32) as ps,
        nc.psum_tensor([128, O], fp32) as ps_scratch,
        nc.semaphore("x_sem") as x_sem,
        nc.semaphore("mm_sem") as mm_sem,
        nc.semaphore("evac_sem") as evac_sem,
        nc.semaphore("y_sem") as y_sem,
        nc.Block() as block,
    ):
        # one semaphore per w chunk: chunks differ in size, so completions on
        # the shared ring can arrive out of order — a cumulative count would
        # unblock matmuls whose own chunk hasn't landed yet
        w_sems = [
            stack.enter_context(nc.semaphore(f"w_sem{k}"))
            for k in range(len(W_CHUNKS))
        ]

        @block.sync
        def _(sync: bass.BassEngine):
            for k, (p, lo, hi) in enumerate(W_CHUNKS):
                if k == 2:
                    # x rides behind the first two w chunks: pair0's weights
                    # land earlier and x is still resident long before pair0
                    # can run
                    sync.dma_start(out=x_sb[:], in_=x_d[:]).then_inc(x_sem, 16)
                if k >= 2:
                    sync.wait_ge(w_sems[k - 2], 16)
                sync.dma_start(
                    out=w_sb[:, p, lo:hi], in_=w_d[:, p, lo:hi]
                ).then_inc(w_sems[k], 16)
            sync.wait_ge(y_sem, 16 * PAIRS)

        @block.scalar
        def _(scalar: bass.BassEngine):
            for p in range(PAIRS):
                scalar.wait_ge(evac_sem, p + 1)
                scalar.dma_start(out=y_d[p], in_=o_sb[:, p, :]).then_inc(y_sem, 16)

        @block.tensor
        def _(tensor: bass.BassEngine):
            # HAM warmup on garbage data while input DMAs stream
            for i in range(WARMUP_MMS):
                tensor.matmul(
                    ps_scratch[0:64, :],
                    dum_x[:, 0:64],
                    dum_w[:],
                    start=(i == 0),
                    stop=(i == WARMUP_MMS - 1),
                )
            tensor.wait_ge(x_sem, 16)
            for p in range(PAIRS):
                if p >= NBANK:
                    tensor.wait_ge(evac_sem, p - NBANK + 1)
                bank = p % NBANK
                last = None
                cur_th = 0
                for dc in range(DC):
                    if thresholds[(p, dc)] > cur_th:
                        cur_th = thresholds[(p, dc)]
                        tensor.wait_ge(w_sems[cur_th - 1], 16)
                    tensor.matmul(
                        ps[0:64, bank, :],
                        x_sb[:, 2 * p, dc, :],
                        w_sb[:, p, dc, 0, :],
                        start=(dc == 0),
                        stop=(dc == DC - 1),
                    )
                    last = tensor.matmul(
                        ps[64:128, bank, :],
                        x_sb[:, 2 * p + 1, dc, :],
                        w_sb[:, p, dc, 1, :],
                        start=(dc == 0),
                        stop=(dc == DC - 1),
                    )
                last.then_inc(mm_sem, 1)

        @block.vector
        def _(vector: bass.BassEngine):
            for p in range(PAIRS):
                vector.wait_ge(mm_sem, p + 1)
                vector.tensor_scalar_mul(
                    o_sb[:, p, :], ps[:, p % NBANK, :], 1.0 / N
                ).then_inc(evac_sem, 1)

    return nc


def _build_program():
    global _program
    if _program is None:
        _program = _build_program_raw()
    return _program


def _prep_inputs(inputs, kernel):
    """Full inputs -> per-core in_maps (fp8 device layouts)."""
    ranks_t = _get_ranks_t()

    # weight bitstreams, directly in device layout [dm, pair, dc, parity, o]
    nb = np.round(np.clip(kernel, 0.0, 1.0) * np.float32(N)).astype(np.int16)  # (O, D)
    nb_m1 = (nb - 1).astype(np.int8).reshape(O, DC, 128).transpose(2, 1, 0)  # [dm,dc,o]
    wb = ranks_t <= nb_m1[:, None, :, None, :]  # bool [dm, pair, dc, parity, o]
    w8 = wb.view(np.uint8) * FP8_ONE

    # input bitstreams [dm, n, dc, b]
    x8 = (inputs.astype(np.uint8) * FP8_ONE).reshape(B, DC, 128, N).transpose(2, 3, 1, 0)

    in_maps = []
    for i in range(NCORES):
        in_maps.append(
            {
                "x": np.ascontiguousarray(x8[:, NPC * i : NPC * (i + 1)]).view(
                    ml_dtypes.float8_e4m3
                ),
                "w": np.ascontiguousarray(w8[:, PAIRS * i : PAIRS * (i + 1)]).view(
                    ml_dtypes.float8_e4m3
                ),
            }
        )
    return in_maps


def _assemble_output(results):
    out = np.empty((B, O, N), np.float32)
    for i, res in enumerate(results):
        y = np.asarray(res["y"])  # (PAIRS, 128, O) fp16
        out[:, :, NPC * i : NPC * (i + 1)] = (
            y.reshape(PAIRS, 2, B, O).transpose(2, 3, 0, 1).reshape(B, O, NPC)
        )
    return out


def run(inputs, kernel, trace=False):
    """Returns (output, BassKernelResults)."""
    from concourse.bass_utils import run_bass_kernel_spmd

    nc = _build_program()
    in_maps = _prep_inputs(np.asarray(inputs), np.asarray(kernel))
    bres = run_bass_kernel_spmd(nc, in_maps, list(range(NCORES)), trace=trace)
    return _assemble_output(bres.results), bres


def kernel(inputs, kernel):
    out, _ = run(inputs, kernel)
    return out
